# revision 7
# baseline (speedup 1.0000x reference)
"""ACE loss kernel for Trainium2, data-parallel over batch across 8 NeuronCores.

Math (matches the reference exactly):
  p[b,c]   = mean_t softmax(preds[t,b,:])[c]
  counts   = per-row histogram of trans_targets (blank bin overwritten with
             T - #positive-valid labels); note sum_c counts[b,c] == T.
  loss     = -sum_{b,c} log(p[b,c]) * counts[b,c] / (B*T)

Device work per core (B_local = 16 batch rows, 64 MB of preds):
  One streaming pass over preds tiles of [128 partitions = (8 t x 16 b), C]:
    ACT: E = exp(x) (bf16) with accum_out -> S[t,b] = sum_c exp (fp32)
    DVE: w = 1/S;  lhsT = mask * w  (block "diagonal" [128, 16], bf16)
    PE : per 512-class chunk, psum[b, n] += sum_p lhsT[p, b] * E[p, n]
         (accumulates the whole T reduction in PSUM across all 16 tiles)
  Epilogue: ACT Ln(psum/T) -> DVE multiply by rearranged counts -> row sums.
Host: tiny histogram of trans_targets, final sum of 8x128 partials.
"""

import sys

sys.path.insert(0, "/opt/trn_rl_repo")

import numpy as np

T, B, C, L = 128, 128, 8192, 50
NCORES = 8
BL = B // NCORES          # 16 batch rows per core
NCH = C // 512            # 16 class chunks of 512
NT = (T * BL) // 128      # 16 row-tiles of 128 (t,b) pairs
BLANK = 0

_CACHE = {}


def _build_nc():
    from concourse import bacc, mybir
    import concourse.tile as tile

    f32 = mybir.dt.float32
    bf16 = mybir.dt.bfloat16
    AF = mybir.ActivationFunctionType

    nc = bacc.Bacc("TRN2", target_bir_lowering=False, debug=False)
    preds = nc.dram_tensor("preds", [T * BL, C], f32, kind="ExternalInput")
    mask = nc.dram_tensor("mask", [128, BL], f32, kind="ExternalInput")
    counts2 = nc.dram_tensor("counts2", [128, 2048], f32, kind="ExternalInput")
    out = nc.dram_tensor("out", [128, 1], f32, kind="ExternalOutput")

    with tile.TileContext(nc) as tc:
        with tc.tile_pool(name="xp", bufs=3) as xp, \
             tc.tile_pool(name="ep", bufs=2) as ep, \
             tc.tile_pool(name="sm", bufs=2) as sm, \
             tc.tile_pool(name="fin", bufs=1) as fin, \
             tc.tile_pool(name="pp", bufs=1, space="PSUM") as pp:
            mask_sb = fin.tile([128, BL], f32, tag="mask")
            nc.sync.dma_start(mask_sb[:], mask.ap())
            cnt_sb = fin.tile([128, 2048], f32, tag="cnt")
            nc.sync.dma_start(cnt_sb[:], counts2.ap())

            # chunk j -> psum bank (j % 8), partition base 32*(j // 8)
            psums = [
                pp.tile([128, 512], f32, tag=f"ps{q}", name=f"ps{q}")
                for q in range(8)
            ]

            for k in range(NT):
                x = xp.tile([128, C], f32, tag="x")
                nc.sync.dma_start(x[:], preds.ap()[k * 128:(k + 1) * 128, :])
                e = ep.tile([128, C], bf16, tag="e")
                s = sm.tile([128, 1], f32, tag="s")
                nc.scalar.activation(e[:], x[:], AF.Exp, accum_out=s[:])
                w = sm.tile([128, 1], f32, tag="w")
                nc.vector.reciprocal(w[:], s[:])
                lh = sm.tile([128, BL], bf16, tag="lh")
                nc.vector.tensor_scalar_mul(lh[:], mask_sb[:], w[:])
                for j in range(NCH):
                    q, r = j % 8, 32 * (j // 8)
                    nc.tensor.matmul(
                        psums[q][r:r + BL, :],
                        lh[:],
                        e[:, j * 512:(j + 1) * 512],
                        start=(k == 0),
                        stop=(k == NT - 1),
                        # Two accumulation groups share each bank on disjoint
                        # partition ranges; the sim's group-region check can't
                        # see the partition split, but the pending-zero value
                        # semantics handle it correctly.
                        skip_group_check=True,
                    )

            # Epilogue: log(P/T), weight by counts, row-sum.
            # Chunk j -> logt partitions [32*(j%4), 32*(j%4)+16) (32-aligned
            # engine partition bases), free block 512*(j//4). The unused 16-row
            # halves are zeroed and their counts are zero.
            logt = fin.tile([128, 2048], f32, tag="logt")
            nc.vector.memset(logt[:], 0.0)
            for j in range(NCH):
                q, r = j % 8, 32 * (j // 8)
                jl, jh = j % 4, j // 4
                nc.scalar.activation(
                    logt[32 * jl:32 * jl + BL, 512 * jh:512 * jh + 512],
                    psums[q][r:r + BL, :],
                    AF.Ln,
                    scale=1.0 / T,
                )
            prod = fin.tile([128, 2048], f32, tag="prod")
            nc.vector.tensor_mul(prod[:], logt[:], cnt_sb[:])
            red = fin.tile([128, 1], f32, tag="red")
            nc.vector.reduce_sum(red[:], prod[:], axis=mybir.AxisListType.X)
            nc.sync.dma_start(out.ap(), red[:])

    nc.compile()
    return nc


def _get_nc():
    if "nc" not in _CACHE:
        _CACHE["nc"] = _build_nc()
    return _CACHE["nc"]


def _host_counts(trans_targets: np.ndarray) -> np.ndarray:
    """Dense [B, C] float32 histogram with the blank-bin overwrite."""
    tt = np.asarray(trans_targets)
    valid = tt < C
    idx = np.where(valid, tt, C).astype(np.int64)
    counts = np.zeros((B, C + 1), np.float32)
    np.add.at(counts, (np.arange(B)[:, None], idx), 1.0)
    counts = counts[:, :C]
    ptl = np.sum(valid & (tt > 0), axis=1)
    counts[:, BLANK] = np.float32(T) - ptl.astype(np.float32)
    return counts


def _make_in_maps(preds: np.ndarray, trans_targets: np.ndarray):
    counts = _host_counts(trans_targets)
    mask = np.zeros((128, BL), np.float32)
    mask[np.arange(128), np.arange(128) % BL] = 1.0
    in_maps = []
    for i in range(NCORES):
        shard = np.ascontiguousarray(
            preds[:, i * BL:(i + 1) * BL, :], dtype=np.float32
        ).reshape(T * BL, C)
        cc = counts[i * BL:(i + 1) * BL]                  # [16, 8192]
        c4 = cc.reshape(BL, 4, 4, 512)                    # [b, jh, jl, n]; j = 4*jh + jl
        c2 = np.zeros((128, 2048), np.float32)
        for jl in range(4):
            c2[32 * jl:32 * jl + BL] = c4[:, :, jl, :].reshape(BL, 2048)
        in_maps.append({"preds": shard, "mask": mask, "counts2": c2})
    return in_maps


def kernel(preds: np.ndarray, trans_targets: np.ndarray) -> np.ndarray:
    from concourse.bass_utils import run_bass_kernel_spmd

    nc = _get_nc()
    in_maps = _make_in_maps(np.asarray(preds), np.asarray(trans_targets))
    res = run_bass_kernel_spmd(
        nc, in_maps, core_ids=list(range(NCORES)),
        trace=bool(_CACHE.get("trace", False)),
    )
    _CACHE["last_exec_ns"] = res.exec_time_ns
    total = np.float64(0.0)
    for core_out in res.results:
        total += np.float64(core_out["out"].sum())
    loss = -total / (np.float64(B) * np.float64(T))
    return np.float32(loss)


# revision 32
# speedup vs baseline: 1.2825x; 1.2825x over previous
"""ACE loss kernel for Trainium2, data-parallel over batch across 8 NeuronCores.

Math (matches the reference exactly):
  p[b,c]   = mean_t softmax(preds[t,b,:])[c]
  counts   = per-row histogram of trans_targets (blank bin overwritten with
             T - #positive-valid labels); note sum_c counts[b,c] == T.
  loss     = -sum_{b,c} log(p[b,c]) * counts[b,c] / (B*T)

Device work per core (B_local = 16 batch rows, 64 MB of preds):
  One streaming pass over preds tiles of [128 partitions = (8 t x 16 b), C],
  each tile split into two 4096-class halves so exp can start on the first
  half while the second half is still in flight:
    ACT: E = exp(x) (bf16) with accum_out -> S[t,b] partial row sums (fp32)
    DVE: S = Sa + Sb; w = 1/S; lhsT = mask * w (block "diagonal" [128,16], bf16)
    PE : per 512-class chunk j, psum[32*(j//8)+b, (j%8)*512+n] += lhsT.T @ E
         (PSUM accumulates the whole T reduction across all 16 tiles)
  Epilogue: 4x ACT Ln(psum/T) -> fused DVE multiply-by-counts + row-sum.
Host: tiny histogram of trans_targets, final sum of 8x128 partials.
"""

import sys

sys.path.insert(0, "/opt/trn_rl_repo")

import numpy as np

T, B, C, L = 128, 128, 8192, 50
NCORES = 8
BL = B // NCORES          # 16 batch rows per core
NCH = C // 512            # 16 class chunks of 512
NT = (T * BL) // 128      # 16 row-tiles of 128 (t,b) pairs
CH = C // 2               # 4096, half-tile class split
BLANK = 0

_CACHE = {}


def _build_nc():
    from concourse import bacc, mybir
    import concourse.tile as tile

    f32 = mybir.dt.float32
    bf16 = mybir.dt.bfloat16
    AF = mybir.ActivationFunctionType
    MUL = mybir.AluOpType.mult

    nc = bacc.Bacc("TRN2", target_bir_lowering=False, debug=False)
    preds = nc.dram_tensor("preds", [T * BL, C], f32, kind="ExternalInput")
    mask = nc.dram_tensor("mask", [128, BL], f32, kind="ExternalInput")
    counts2 = nc.dram_tensor("counts2", [128, 2048], f32, kind="ExternalInput")
    out = nc.dram_tensor("out", [1, 1], f32, kind="ExternalOutput")

    with tile.TileContext(nc) as tc:
        with tc.tile_pool(name="xp", bufs=3) as xp, \
             tc.tile_pool(name="xhp", bufs=4) as xhp, \
             tc.tile_pool(name="ep", bufs=3) as ep, \
             tc.tile_pool(name="sm", bufs=2) as sm, \
             tc.tile_pool(name="fin", bufs=1) as fin, \
             tc.tile_pool(name="pp", bufs=1, space="PSUM") as pp:
            # Preload the ACT table set that holds BOTH Exp and Ln
            # (natural_log_exp_and_others, id 6) so no table switch lands on
            # the critical tail between the last exp and the Ln epilogue.
            nc.scalar.add_instruction(mybir.InstLoadActFuncSet(
                name=nc.get_next_instruction_name(), ins=[], outs=[],
                act_func_set_id=6))

            # One PSUM tensor spanning all 8 banks; chunk j lives at
            # partitions [32*(j//8), +16), free bytes of bank (j % 8).
            psum_t = pp.tile([128, 4096], f32, tag="psum", name="psum_t")

            # Small inputs + epilogue tiles (memset runs early, off the
            # DMA-critical path).
            mask_sb = fin.tile([128, BL], f32, tag="mask")
            cnt_sb = fin.tile([128, 2048], f32, tag="cnt")
            logt = fin.tile([128, 2048], f32, tag="logt")
            ones = fin.tile([128, 1], f32, tag="ones")

            def emit_ln(pg, bp):
                # Ln over bank pair bp (banks 2bp, 2bp+1), partition group pg:
                # chunks {8*pg + 2*bp, 8*pg + 2*bp + 1} -> classes
                # [(8*pg + 2*bp)*512, +1024). logt partition group
                # g = 2*pg + (bp // 2), columns (bp % 2)*1024.
                g = 2 * pg + (bp // 2)
                nc.scalar.activation(
                    logt[32 * g:32 * g + BL,
                         (bp % 2) * 1024:(bp % 2) * 1024 + 1024],
                    psum_t[32 * pg:32 * pg + BL, bp * 1024:bp * 1024 + 1024],
                    AF.Ln,
                    scale=1.0 / T,
                )

            for k in range(NT):
                last = k == NT - 1
                if not last:
                    x = xp.tile([128, C], f32, tag="x", name="x")
                    nc.sync.dma_start(x[:], preds.ap()[k * 128:(k + 1) * 128, :])
                else:
                    # Split the final tile into quarters so only a quarter-exp
                    # sits on the critical tail after the last DMA completes.
                    rows = preds.ap()[k * 128:(k + 1) * 128, :]
                    xq = []
                    for q in range(4):
                        t = xhp.tile([128, C // 4], f32, tag="xh", name=f"xq{q}")
                        nc.sync.dma_start(t[:], rows[:, q * (C // 4):(q + 1) * (C // 4)])
                        xq.append(t)
                if k == 0:
                    nc.sync.dma_start(mask_sb[:], mask.ap())
                    nc.sync.dma_start(cnt_sb[:], counts2.ap())
                    nc.vector.memset(logt[:], 0.0)
                    nc.vector.memset(ones[:], 1.0)
                e = ep.tile([128, C], bf16, tag="e", name="e")
                w = sm.tile([128, 1], f32, tag="w")
                if not last:
                    s = sm.tile([128, 1], f32, tag="s")
                    nc.scalar.activation(e[:], x[:], AF.Exp, accum_out=s[:])
                    nc.vector.reciprocal(w[:], s[:])
                else:
                    CQ = C // 4
                    sq = [sm.tile([128, 1], f32, tag=f"sq{q}", name=f"sq{q}")
                          for q in range(4)]
                    for q in range(4):
                        nc.scalar.activation(
                            e[:, q * CQ:(q + 1) * CQ], xq[q][:], AF.Exp,
                            accum_out=sq[q][:])
                    nc.vector.tensor_add(w[:], sq[0][:], sq[1][:])
                    nc.vector.tensor_add(sq[2][:], sq[2][:], sq[3][:])
                    nc.vector.tensor_add(w[:], w[:], sq[2][:])
                    nc.vector.reciprocal(w[:], w[:])
                lh = sm.tile([128, BL], bf16, tag="lh")
                nc.vector.tensor_scalar_mul(lh[:], mask_sb[:], w[:])
                # In the last iteration, order matmuls bank-pair by bank-pair
                # so each pair's Ln can start while later banks accumulate.
                jorder = (
                    list(range(NCH)) if not last
                    else [0, 8, 1, 9, 2, 10, 3, 11, 4, 12, 5, 13, 6, 14, 7, 15]
                )
                for jj, j in enumerate(jorder):
                    nc.tensor.matmul(
                        psum_t[32 * (j // 8):32 * (j // 8) + BL,
                               (j % 8) * 512:(j % 8) * 512 + 512],
                        lh[:],
                        e[:, j * 512:(j + 1) * 512],
                        start=(k == 0),
                        stop=last,
                        # Two accumulation groups share each bank on disjoint
                        # partition ranges; the sim's group-region check can't
                        # see the partition split, but the pending-zero value
                        # semantics handle it correctly.
                        skip_group_check=True,
                    )
                    if last and jj % 4 == 3:
                        bp = jj // 4
                        emit_ln(0, bp)
                        emit_ln(1, bp)

            # Epilogue: fused multiply-by-counts + row-sum.
            prod = fin.tile([128, 2048], f32, tag="prod")
            red = fin.tile([128, 1], f32, tag="red")
            nc.vector.scalar_tensor_tensor(
                prod[:], logt[:], 1.0, cnt_sb[:], op0=MUL, op1=MUL,
                accum_out=red[:],
            )
            # Collapse the 128 per-partition partials to one scalar with a
            # ones-matmul so the result DMA is a single 4-byte descriptor
            # (a [128,1] store scatters 128 4-byte writes and costs ~7us).
            nc.tensor.matmul(psum_t[0:1, 0:1], ones[:], red[:],
                             start=True, stop=True, skip_group_check=True)
            final = fin.tile([1, 1], f32, tag="final")
            nc.vector.tensor_copy(final[:], psum_t[0:1, 0:1])
            nc.gpsimd.dma_start(out.ap(), final[:])

    nc.compile()
    return nc


def _get_nc():
    if "nc" not in _CACHE:
        _CACHE["nc"] = _build_nc()
    return _CACHE["nc"]


def _host_counts(trans_targets: np.ndarray) -> np.ndarray:
    """Dense [B, C] float32 histogram with the blank-bin overwrite."""
    tt = np.asarray(trans_targets)
    valid = tt < C
    idx = np.where(valid, tt, C).astype(np.int64)
    counts = np.zeros((B, C + 1), np.float32)
    np.add.at(counts, (np.arange(B)[:, None], idx), 1.0)
    counts = counts[:, :C]
    ptl = np.sum(valid & (tt > 0), axis=1)
    counts[:, BLANK] = np.float32(T) - ptl.astype(np.float32)
    return counts


def _make_in_maps(preds: np.ndarray, trans_targets: np.ndarray):
    counts = _host_counts(trans_targets)
    mask = np.zeros((128, BL), np.float32)
    mask[np.arange(128), np.arange(128) % BL] = 1.0
    in_maps = []
    for i in range(NCORES):
        shard = np.ascontiguousarray(
            preds[:, i * BL:(i + 1) * BL, :], dtype=np.float32
        ).reshape(T * BL, C)
        cc = counts[i * BL:(i + 1) * BL]                  # [16, 8192]
        c3 = cc.reshape(BL, 4, 2048)                      # [b, g, n]
        c2 = np.zeros((128, 2048), np.float32)
        for g in range(4):
            c2[32 * g:32 * g + BL] = c3[:, g, :]
        in_maps.append({"preds": shard, "mask": mask, "counts2": c2})
    return in_maps


def kernel(preds: np.ndarray, trans_targets: np.ndarray) -> np.ndarray:
    from concourse.bass_utils import run_bass_kernel_spmd

    nc = _get_nc()
    in_maps = _make_in_maps(np.asarray(preds), np.asarray(trans_targets))
    res = run_bass_kernel_spmd(
        nc, in_maps, core_ids=list(range(NCORES)),
        trace=bool(_CACHE.get("trace", False)),
    )
    _CACHE["last_exec_ns"] = res.exec_time_ns
    _CACHE["last_res"] = res
    total = np.float64(0.0)
    for core_out in res.results:
        total += np.float64(core_out["out"].sum())
    loss = -total / (np.float64(B) * np.float64(T))
    return np.float32(loss)
